# revision 30
# baseline (speedup 1.0000x reference)
"""Trainium2 Bass kernel for single-head attention (B=8, S=2048, DIN=768, DOUT=64).

Strategy: data parallel — one batch element per NeuronCore (8 cores).
Per core, attention runs in transposed-score layout (k on partitions, q on
free dim), ScalarE-paced at ~1.11us per [128,1024] exp. All math is bf16
with fp32 PSUM accumulation (fp8/DoubleRow measures 2 cyc/col on this HW —
no gain — so bf16 at 1 col/cycle is optimal).

  warmup    scratch matmuls bridge every prologue gap so the PE HAM clock
            gate latches 8/8 (2.4 GHz) and the main loop never idles long
            enough (>3.4us) to drop back to 1.2 GHz
  qk proj   [Wq|Wk] combined: 6 chunk matmuls per 512-col block ->
            PSUM [q|k, cols]; DVE splits into qT/kT (bias added)
  scores    kT-tile stationary [64,128], qT moving -> sc[k,q] PSUM
  exp       ScalarE activation, PSUM fp32 -> SBUF bf16 (the pace-setter:
            32 x [128,1024] = ~35.6us busy)
  mask      DVE multiply by keep (=~mask) bf16, 2x_1p mode (~0.7us/unit)
  ctx       bf16 matmul, v65 = [v | 1] stationary (row 64 = softmax denom)
  epilogue  PE transposes + reciprocal + scale, interleaved into pass 1

Loop is q-half-major (2 passes x 16 k-tiles) so ctx PSUM is 2 banks and the
score PSUM triple-buffers (3x2 banks): 3*2 + 2 = 8 banks, keeping ScalarE
gap-free. v projections are interleaved into pass 0 so their LDWEIGHTS hide
under main-loop matmuls.
"""

import math
import sys
from contextlib import ExitStack

import numpy as np

sys.path.insert(0, "/opt/trn_rl_repo")

import ml_dtypes  # noqa: E402

import concourse.bass as bass  # noqa: E402
import concourse.tile as tile  # noqa: E402
from concourse import bacc, mybir  # noqa: E402
from concourse.bass import ds  # noqa: E402
from concourse.bass_utils import run_bass_kernel_spmd  # noqa: E402
from concourse.masks import make_identity  # noqa: E402

B, S, DIN, DOUT = 8, 2048, 768, 64
P = 128
NCH = 6  # din chunks
KT = S // P  # 16 key tiles
NB = 4  # qk projection column blocks of 512
NS = 512  # matmul moving free dim (one PSUM bank fp32)
H = 2  # q halves (passes)
HQ = S // H  # 1024

F32 = mybir.dt.float32
BF16 = mybir.dt.bfloat16

_NC_CACHE = None


def build_nc():
    nc = bacc.Bacc("TRN2", target_bir_lowering=False, debug=False)

    xT = nc.declare_dram_parameter("xT", [NB, NCH, P, NS], BF16, isOutput=False)
    keep = nc.declare_dram_parameter("keep", [KT, P, S], BF16, isOutput=False)
    wqk = nc.declare_dram_parameter("wqk", [NCH, P, P], BF16, isOutput=False)
    wv = nc.declare_dram_parameter("wv", [NCH, P, DOUT], BF16, isOutput=False)
    bqk = nc.declare_dram_parameter("bqk", [P, 1], F32, isOutput=False)
    out = nc.declare_dram_parameter("out", [S, DOUT + 1], BF16, isOutput=True)

    inv_sqrt_s = float(1.0 / math.sqrt(S))

    with tile.TileContext(nc) as tc, ExitStack() as ctx:
        singles = ctx.enter_context(tc.tile_pool(name="singles", bufs=1))
        epool = ctx.enter_context(tc.tile_pool(name="epool", bufs=3))
        opool = ctx.enter_context(tc.tile_pool(name="opool", bufs=4))

        # ---- constants / weights (small DMAs first)
        wqk_sb = singles.tile([P, NCH, P], BF16)
        nc.sync.dma_start(out=wqk_sb, in_=wqk.rearrange("c p m -> p c m"))
        wv_sb = singles.tile([P, NCH, DOUT], BF16)
        nc.sync.dma_start(out=wv_sb, in_=wv.rearrange("c p m -> p c m"))
        bqk_sb = singles.tile([P, 1], F32)
        nc.sync.dma_start(out=bqk_sb, in_=bqk[:, :])

        # ---- big inputs, in consumption-priority order: x blocks 0-1
        # (gate the first exp), keep pass-0 halves for early tiles, x blocks
        # 2-3, the rest of keep pass-0, then all keep pass-1 halves.
        xT_sb = singles.tile([P, NCH, S], BF16)
        keep_sb = singles.tile([P, KT, S], BF16)

        def dma_x_block(blk):
            nc.sync.dma_start(
                out=xT_sb[:, :, ds(blk * NS, NS)],
                in_=xT[blk].rearrange("c p s -> p c s"),
            )

        def dma_keep_half(t, h):
            nc.sync.dma_start(
                out=keep_sb[:, t, ds(h * HQ, HQ)], in_=keep[t, :, ds(h * HQ, HQ)]
            )

        dma_x_block(0)
        dma_x_block(1)
        dma_keep_half(0, 0)
        dma_keep_half(1, 0)
        dma_keep_half(2, 0)
        dma_x_block(2)
        dma_keep_half(3, 0)
        dma_keep_half(4, 0)
        dma_x_block(3)
        for t in range(5, KT):
            dma_keep_half(t, 0)
        for t in range(KT):
            dma_keep_half(t, 1)

        ident = singles.tile([P, P], F32)
        make_identity(nc, ident)
        ident_bf = singles.tile([P, P], BF16)
        make_identity(nc, ident_bf)

        # ---- v with a ones column: [s(128 part), ktile, 65] bf16
        v65_sb = singles.tile([P, KT, DOUT + 1], BF16)
        nc.gpsimd.memset(v65_sb, 1.0)

        warm_sb = singles.tile([P, NS], BF16)
        nc.gpsimd.memset(warm_sb, 0.0)

        qT_sb = singles.tile([DOUT, S], BF16)
        kT_sb = singles.tile([DOUT, S], BF16)
        ctxT_sb = singles.tile([DOUT + 1, S], BF16)
        ctxT2 = singles.tile([DOUT + 1, HQ], BF16)

        with (
            tc.tile_pool(name="psS", bufs=3, space="PSUM") as psS,
            tc.tile_pool(name="psC", bufs=1, space="PSUM") as psC,
        ):

            def filler(n):
                # prologue-only scratch matmuls to hold the HAM gate at 8/8
                wps = psS.tile([P, HQ], F32, tag="big", name="wps")
                for _ in range(n):
                    nc.tensor.matmul(
                        wps[:, 0:NS], lhsT=warm_sb[:, 0:P], rhs=warm_sb[:, 0:NS],
                        start=True, stop=True, skip_group_check=True,
                    )

            def vproj_mms(t):
                v_ps = psS.tile([P, HQ], F32, tag="big", name="v_ps")
                for c in range(NCH):
                    nc.tensor.matmul(
                        v_ps[:, 0:DOUT],
                        lhsT=xT_sb[:, c, ds(t * P, P)],
                        rhs=wv_sb[:, c, :],
                        start=(c == 0),
                        stop=(c == NCH - 1),
                    )
                return v_ps

            def vproj_copy(t, v_ps):
                nc.vector.tensor_copy(v65_sb[:, t, 0:DOUT], v_ps[:, 0:DOUT])

            def vproj(t):
                vproj_copy(t, vproj_mms(t))

            # ---- HAM warmup while the first x blocks stream in
            filler(12)

            # ---- qk projection + split into qT/kT (bias added on DVE);
            # blocks 0-1 gate the first exp, blocks 2-3 run lazily inside
            # early pass-0 units (first needed by k-tile 8)
            def qk_mms(blk, qk_ps=None, chunks=range(NCH)):
                if qk_ps is None:
                    qk_ps = psS.tile([P, HQ], F32, tag="big", name="qk_ps")
                for c in chunks:
                    nc.tensor.matmul(
                        qk_ps[:, 0:NS],
                        lhsT=wqk_sb[:, c, :],
                        rhs=xT_sb[:, c, ds(blk * NS, NS)],
                        start=(c == 0),
                        stop=(c == NCH - 1),
                    )
                return qk_ps

            def qk_splits(blk, qk_ps):
                cols = ds(blk * NS, NS)
                nc.vector.tensor_scalar_add(
                    qT_sb[:, cols], qk_ps[0:DOUT, 0:NS], bqk_sb[0:DOUT]
                )
                nc.vector.tensor_scalar_add(
                    kT_sb[:, cols], qk_ps[DOUT:P, 0:NS], bqk_sb[DOUT:P]
                )

            def qk_block(blk):
                qk_splits(blk, qk_mms(blk))

            qk_block(0)
            filler(3)
            qk_block(1)
            filler(2)

            # ---- epilogue worker: transpose a 4-tile group of the
            # numerator|denominator columns and ship unnormalized [q, 65]
            # (the host divides by column 64 -- free off-chip)
            D1 = DOUT + 1

            def epilogue_half(h):
                ogs = []
                for g in range(2):
                    tr = psS.tile([P, 2 * HQ], BF16, tag="big", name="trb")
                    for gi in range(4):
                        qt = g * 4 + gi
                        csrc = ctxT_sb if h == 0 else ctxT2
                        nc.tensor.transpose(
                            tr[:, ds(gi * (D1 + 1), D1)],
                            csrc[:, ds(qt * P, P)],
                            ident_bf[0:D1, 0:D1],
                        )
                    og = opool.tile([P, 4, D1], BF16, tag="ostage")
                    nc.vector.tensor_copy(
                        og,
                        tr[:, 0 : 4 * (D1 + 1)].rearrange(
                            "p (t m) -> p t m", m=D1 + 1
                        )[:, :, 0:D1],
                    )
                    ogs.append(og)
                for g, og in enumerate(ogs):
                    nc.sync.dma_start(
                        out=out[ds(h * HQ + g * 4 * P, 4 * P), :].rearrange(
                            "(t p) m -> p t m", p=P
                        ),
                        in_=og,
                    )

            # ---- main loop: q-half-major, 16 k-tiles inside
            for h in range(H):
                ctx_ps = psC.tile([DOUT + 1, HQ], F32)
                for t in range(KT):
                    sc = psS.tile([P, HQ], F32, tag="big")
                    for n in range(HQ // NS):
                        nc.tensor.matmul(
                            sc[:, ds(n * NS, NS)],
                            lhsT=kT_sb[:, ds(t * P, P)],
                            rhs=qT_sb[:, ds(h * HQ + n * NS, NS)],
                            start=True,
                            stop=True,
                        )
                    ex = epool.tile([P, HQ], BF16, tag="exp")
                    nc.scalar.activation(
                        out=ex,
                        in_=sc,
                        func=mybir.ActivationFunctionType.Exp,
                        scale=inv_sqrt_s,
                    )
                    # PE extras run in the window where ctx waits on the exp
                    v_ps = qk_ps = v_ps0 = None
                    if h == 0 and t == 0:
                        v_ps0 = vproj_mms(0)
                    if h == 0 and t < KT - 1:
                        v_ps = vproj_mms(t + 1)
                    if h == 0 and t in (5, 9):
                        qk_hold = qk_mms(2 + (t - 5) // 4, chunks=range(0, 3))
                    if h == 0 and t in (6, 10):
                        qk_ps = qk_mms(
                            2 + (t - 6) // 4, qk_ps=qk_hold, chunks=range(3, NCH)
                        )
                    if h == 0 and t == KT - 1:
                        filler(4)
                    nc.vector.tensor_mul(ex, ex, keep_sb[:, t, ds(h * HQ, HQ)])
                    if v_ps0 is not None:
                        vproj_copy(0, v_ps0)
                    if v_ps is not None:
                        vproj_copy(t + 1, v_ps)
                    if qk_ps is not None:
                        qk_splits(2 + (t - 6) // 4, qk_ps)
                    for n in range(HQ // NS):
                        nc.tensor.matmul(
                            ctx_ps[:, ds(n * NS, NS)],
                            lhsT=v65_sb[:, t, :],
                            rhs=ex[:, ds(n * NS, NS)],
                            start=(t == 0),
                            stop=(t == KT - 1),
                        )

                if h == 0:
                    nc.vector.tensor_copy(ctxT_sb[:, ds(0, HQ)], ctx_ps)
                else:
                    epilogue_half(0)
                    nc.vector.tensor_copy(ctxT2, ctx_ps)
                    epilogue_half(1)

    nc.finalize()
    return nc


def _get_nc():
    global _NC_CACHE
    if _NC_CACHE is None:
        _NC_CACHE = build_nc()
    return _NC_CACHE


def kernel(**inputs):
    x = np.asarray(inputs["input_tensor"], dtype=np.float32)  # [B, S, DIN]
    mask = np.asarray(inputs["attention_mask"])  # [B, S, S] bool
    Wq = np.asarray(inputs["Wq"], dtype=np.float32)
    Wk = np.asarray(inputs["Wk"], dtype=np.float32)
    Wv = np.asarray(inputs["Wv"], dtype=np.float32)
    bq = np.asarray(inputs["bq"], dtype=np.float32)
    bk = np.asarray(inputs["bk"], dtype=np.float32)
    bv = np.asarray(inputs["bv"], dtype=np.float32)

    Wqk = np.concatenate([Wq, Wk], axis=1)  # [768, 128]
    wqk_h = np.ascontiguousarray(Wqk.reshape(NCH, P, P)).astype(ml_dtypes.bfloat16)
    wv_h = np.ascontiguousarray(Wv.reshape(NCH, P, DOUT)).astype(ml_dtypes.bfloat16)
    bqk_h = np.ascontiguousarray(np.concatenate([bq, bk]).reshape(P, 1))

    in_maps = []
    for b in range(B):
        xTb = np.ascontiguousarray(x[b].T)  # [DIN, S] fp32
        xT_h = np.ascontiguousarray(
            xTb.reshape(NCH, P, NB, NS).transpose(2, 0, 1, 3)
        ).astype(ml_dtypes.bfloat16)
        # keep = ~mask, transposed to [k, q], per key tile
        keepT = (~mask[b]).T
        keep_h = np.ascontiguousarray(keepT.reshape(KT, P, S)).astype(
            ml_dtypes.bfloat16
        )
        in_maps.append(
            {
                "xT": xT_h,
                "keep": keep_h,
                "wqk": wqk_h,
                "wv": wv_h,
                "bqk": bqk_h,
            }
        )

    nc = _get_nc()
    res = run_bass_kernel_spmd(nc, in_maps, core_ids=list(range(B)))
    raw = np.stack(
        [np.asarray(res.results[b]["out"]).astype(np.float32) for b in range(B)]
    )
    out = raw[:, :, :DOUT] / raw[:, :, DOUT:]  # normalize by the softmax denom
    out = out + bv[None, None, :]
    return out.astype(np.float32)


# revision 31
# speedup vs baseline: 1.0247x; 1.0247x over previous
"""Trainium2 Bass kernel for single-head attention (B=8, S=2048, DIN=768, DOUT=64).

Strategy: data parallel — one batch element per NeuronCore (8 cores).
Per core, attention runs in transposed-score layout (k on partitions, q on
free dim), ScalarE-paced at ~1.11us per [128,1024] exp. All math is bf16
with fp32 PSUM accumulation (fp8/DoubleRow measures 2 cyc/col on this HW —
no gain — so bf16 at 1 col/cycle is optimal).

  warmup    scratch matmuls bridge every prologue gap so the PE HAM clock
            gate latches 8/8 (2.4 GHz) and the main loop never idles long
            enough (>3.4us) to drop back to 1.2 GHz
  qk proj   [Wq|Wk] combined: 6 chunk matmuls per 512-col block ->
            PSUM [q|k, cols]; DVE splits into qT/kT (bias added)
  scores    kT-tile stationary [64,128], qT moving -> sc[k,q] PSUM
  exp       ScalarE activation, PSUM fp32 -> SBUF bf16 (the pace-setter:
            32 x [128,1024] = ~35.6us busy)
  mask      DVE multiply by keep (=~mask) bf16, 2x_1p mode (~0.7us/unit)
  ctx       bf16 matmul, v65 = [v | 1] stationary (row 64 = softmax denom)
  epilogue  PE transposes + reciprocal + scale, interleaved into pass 1

Loop is q-half-major (2 passes x 16 k-tiles) so ctx PSUM is 2 banks and the
score PSUM triple-buffers (3x2 banks): 3*2 + 2 = 8 banks, keeping ScalarE
gap-free. v projections are interleaved into pass 0 so their LDWEIGHTS hide
under main-loop matmuls.
"""

import math
import sys
from contextlib import ExitStack

import numpy as np

sys.path.insert(0, "/opt/trn_rl_repo")

import ml_dtypes  # noqa: E402

import concourse.bass as bass  # noqa: E402
import concourse.tile as tile  # noqa: E402
from concourse import bacc, mybir  # noqa: E402
from concourse.bass import ds  # noqa: E402
from concourse.bass_utils import run_bass_kernel_spmd  # noqa: E402
from concourse.masks import make_identity  # noqa: E402

B, S, DIN, DOUT = 8, 2048, 768, 64
P = 128
NCH = 6  # din chunks
KT = S // P  # 16 key tiles
NB = 4  # qk projection column blocks of 512
NS = 512  # matmul moving free dim (one PSUM bank fp32)
H = 2  # q halves (passes)
HQ = S // H  # 1024

F32 = mybir.dt.float32
BF16 = mybir.dt.bfloat16

_NC_CACHE = None


def build_nc():
    nc = bacc.Bacc("TRN2", target_bir_lowering=False, debug=False)

    xT = nc.declare_dram_parameter("xT", [NB, NCH, P, NS], BF16, isOutput=False)
    keep = nc.declare_dram_parameter("keep", [KT, P, S], BF16, isOutput=False)
    wqk = nc.declare_dram_parameter("wqk", [NCH, P, P], BF16, isOutput=False)
    wv = nc.declare_dram_parameter("wv", [NCH, P, DOUT], BF16, isOutput=False)
    bqk = nc.declare_dram_parameter("bqk", [P, 1], F32, isOutput=False)
    out = nc.declare_dram_parameter("out", [S, DOUT + 1], BF16, isOutput=True)

    inv_sqrt_s = float(1.0 / math.sqrt(S))

    with tile.TileContext(nc) as tc, ExitStack() as ctx:
        singles = ctx.enter_context(tc.tile_pool(name="singles", bufs=1))
        epool = ctx.enter_context(tc.tile_pool(name="epool", bufs=3))
        opool = ctx.enter_context(tc.tile_pool(name="opool", bufs=4))

        # ---- constants / weights (small DMAs first)
        wqk_sb = singles.tile([P, NCH, P], BF16)
        nc.sync.dma_start(out=wqk_sb, in_=wqk.rearrange("c p m -> p c m"))
        wv_sb = singles.tile([P, NCH, DOUT], BF16)
        nc.sync.dma_start(out=wv_sb, in_=wv.rearrange("c p m -> p c m"))
        bqk_sb = singles.tile([P, 1], F32)
        nc.sync.dma_start(out=bqk_sb, in_=bqk[:, :])

        # ---- big inputs, in consumption-priority order: x blocks 0-1
        # (gate the first exp), keep pass-0 halves for early tiles, x blocks
        # 2-3, the rest of keep pass-0, then all keep pass-1 halves.
        xT_sb = singles.tile([P, NCH, S], BF16)
        keep_sb = singles.tile([P, KT, S], BF16)

        def dma_x_block(blk):
            nc.sync.dma_start(
                out=xT_sb[:, :, ds(blk * NS, NS)],
                in_=xT[blk].rearrange("c p s -> p c s"),
            )

        def dma_keep_half(t, h):
            nc.sync.dma_start(
                out=keep_sb[:, t, ds(h * HQ, HQ)], in_=keep[t, :, ds(h * HQ, HQ)]
            )

        dma_x_block(0)
        dma_x_block(1)
        dma_keep_half(0, 0)
        dma_keep_half(1, 0)
        dma_keep_half(2, 0)
        dma_x_block(2)
        dma_keep_half(3, 0)
        dma_keep_half(4, 0)
        dma_x_block(3)
        for t in range(5, KT):
            dma_keep_half(t, 0)
        for t in range(KT):
            dma_keep_half(t, 1)

        ident = singles.tile([P, P], F32)
        make_identity(nc, ident)
        ident_bf = singles.tile([P, P], BF16)
        make_identity(nc, ident_bf)

        # ---- v with a ones column: [s(128 part), ktile, 65] bf16
        v65_sb = singles.tile([P, KT, DOUT + 1], BF16)
        nc.gpsimd.memset(v65_sb, 1.0)

        warm_sb = singles.tile([P, NS], BF16)
        nc.gpsimd.memset(warm_sb, 0.0)

        qT_sb = singles.tile([DOUT, S], BF16)
        kT_sb = singles.tile([DOUT, S], BF16)
        ctxT_sb = singles.tile([DOUT + 1, S], BF16)
        ctxT2 = singles.tile([DOUT + 1, HQ], BF16)

        with (
            tc.tile_pool(name="psS", bufs=3, space="PSUM") as psS,
            tc.tile_pool(name="psC", bufs=1, space="PSUM") as psC,
        ):

            def filler(n):
                # prologue-only scratch matmuls to hold the HAM gate at 8/8
                wps = psS.tile([P, HQ], F32, tag="big", name="wps")
                for _ in range(n):
                    nc.tensor.matmul(
                        wps[:, 0:NS], lhsT=warm_sb[:, 0:P], rhs=warm_sb[:, 0:NS],
                        start=True, stop=True, skip_group_check=True,
                    )

            def vproj_mms(t):
                v_ps = psS.tile([P, HQ], F32, tag="big", name="v_ps")
                for c in range(NCH):
                    nc.tensor.matmul(
                        v_ps[:, 0:DOUT],
                        lhsT=xT_sb[:, c, ds(t * P, P)],
                        rhs=wv_sb[:, c, :],
                        start=(c == 0),
                        stop=(c == NCH - 1),
                    )
                return v_ps

            def vproj_copy(t, v_ps):
                nc.vector.tensor_copy(v65_sb[:, t, 0:DOUT], v_ps[:, 0:DOUT])

            def vproj(t):
                vproj_copy(t, vproj_mms(t))

            # ---- HAM warmup while the first x blocks stream in
            filler(12)

            # ---- qk projection + split into qT/kT (bias added on DVE);
            # blocks 0-1 gate the first exp, blocks 2-3 run lazily inside
            # early pass-0 units (first needed by k-tile 8)
            def qk_mms(blk, qk_ps=None, chunks=range(NCH)):
                if qk_ps is None:
                    qk_ps = psS.tile([P, HQ], F32, tag="big", name="qk_ps")
                for c in chunks:
                    nc.tensor.matmul(
                        qk_ps[:, 0:NS],
                        lhsT=wqk_sb[:, c, :],
                        rhs=xT_sb[:, c, ds(blk * NS, NS)],
                        start=(c == 0),
                        stop=(c == NCH - 1),
                    )
                return qk_ps

            def qk_splits(blk, qk_ps):
                cols = ds(blk * NS, NS)
                nc.vector.tensor_scalar_add(
                    qT_sb[:, cols], qk_ps[0:DOUT, 0:NS], bqk_sb[0:DOUT]
                )
                nc.vector.tensor_scalar_add(
                    kT_sb[:, cols], qk_ps[DOUT:P, 0:NS], bqk_sb[DOUT:P]
                )

            def qk_block(blk):
                qk_splits(blk, qk_mms(blk))

            qk_block(0)
            filler(3)
            qk_block(1)
            filler(2)

            # ---- epilogue worker: transpose a 4-tile group of the
            # numerator|denominator columns and ship unnormalized [q, 65]
            # (the host divides by column 64 -- free off-chip)
            D1 = DOUT + 1

            def epilogue_half(h):
                ogs = []
                for g in range(2):
                    tr = psS.tile([P, 2 * HQ], BF16, tag="big", name="trb")
                    for gi in range(4):
                        qt = g * 4 + gi
                        csrc = ctxT_sb if h == 0 else ctxT2
                        nc.tensor.transpose(
                            tr[:, ds(gi * (D1 + 1), D1)],
                            csrc[:, ds(qt * P, P)],
                            ident_bf[0:D1, 0:D1],
                        )
                    og = opool.tile([P, 4, D1], BF16, tag="ostage")
                    nc.vector.tensor_copy(
                        og,
                        tr[:, 0 : 4 * (D1 + 1)].rearrange(
                            "p (t m) -> p t m", m=D1 + 1
                        )[:, :, 0:D1],
                    )
                    ogs.append(og)
                for g, og in enumerate(ogs):
                    nc.sync.dma_start(
                        out=out[ds(h * HQ + g * 4 * P, 4 * P), :].rearrange(
                            "(t p) m -> p t m", p=P
                        ),
                        in_=og,
                    )

            # ---- main loop: q-half-major, 16 k-tiles inside
            for h in range(H):
                ctx_ps = psC.tile([DOUT + 1, HQ], F32)
                for t in range(KT):
                    sc = psS.tile([P, HQ], F32, tag="big")
                    for n in range(HQ // NS):
                        nc.tensor.matmul(
                            sc[:, ds(n * NS, NS)],
                            lhsT=kT_sb[:, ds(t * P, P)],
                            rhs=qT_sb[:, ds(h * HQ + n * NS, NS)],
                            start=True,
                            stop=True,
                        )
                    ex = epool.tile([P, HQ], BF16, tag="exp")
                    nc.scalar.activation(
                        out=ex,
                        in_=sc,
                        func=mybir.ActivationFunctionType.Exp,
                        scale=inv_sqrt_s,
                    )
                    # PE extras run in the window where ctx waits on the exp
                    v_ps = qk_ps = v_ps0 = None
                    if h == 0 and t == 0:
                        v_ps0 = vproj_mms(0)
                    if h == 0 and t < KT - 1:
                        v_ps = vproj_mms(t + 1)
                    if h == 0 and t in (6, 10):
                        qk_ps = qk_mms(2 + (t - 6) // 4)
                    if h == 0 and t == KT - 1:
                        filler(4)
                    nc.vector.tensor_mul(ex, ex, keep_sb[:, t, ds(h * HQ, HQ)])
                    if v_ps0 is not None:
                        vproj_copy(0, v_ps0)
                    if v_ps is not None:
                        vproj_copy(t + 1, v_ps)
                    if qk_ps is not None:
                        qk_splits(2 + (t - 6) // 4, qk_ps)
                    for n in range(HQ // NS):
                        nc.tensor.matmul(
                            ctx_ps[:, ds(n * NS, NS)],
                            lhsT=v65_sb[:, t, :],
                            rhs=ex[:, ds(n * NS, NS)],
                            start=(t == 0),
                            stop=(t == KT - 1),
                        )

                if h == 0:
                    nc.vector.tensor_copy(ctxT_sb[:, ds(0, HQ)], ctx_ps)
                else:
                    epilogue_half(0)
                    nc.vector.tensor_copy(ctxT2, ctx_ps)
                    epilogue_half(1)

    nc.finalize()
    return nc


def _get_nc():
    global _NC_CACHE
    if _NC_CACHE is None:
        _NC_CACHE = build_nc()
    return _NC_CACHE


def kernel(**inputs):
    x = np.asarray(inputs["input_tensor"], dtype=np.float32)  # [B, S, DIN]
    mask = np.asarray(inputs["attention_mask"])  # [B, S, S] bool
    Wq = np.asarray(inputs["Wq"], dtype=np.float32)
    Wk = np.asarray(inputs["Wk"], dtype=np.float32)
    Wv = np.asarray(inputs["Wv"], dtype=np.float32)
    bq = np.asarray(inputs["bq"], dtype=np.float32)
    bk = np.asarray(inputs["bk"], dtype=np.float32)
    bv = np.asarray(inputs["bv"], dtype=np.float32)

    Wqk = np.concatenate([Wq, Wk], axis=1)  # [768, 128]
    wqk_h = np.ascontiguousarray(Wqk.reshape(NCH, P, P)).astype(ml_dtypes.bfloat16)
    wv_h = np.ascontiguousarray(Wv.reshape(NCH, P, DOUT)).astype(ml_dtypes.bfloat16)
    bqk_h = np.ascontiguousarray(np.concatenate([bq, bk]).reshape(P, 1))

    in_maps = []
    for b in range(B):
        xTb = np.ascontiguousarray(x[b].T)  # [DIN, S] fp32
        xT_h = np.ascontiguousarray(
            xTb.reshape(NCH, P, NB, NS).transpose(2, 0, 1, 3)
        ).astype(ml_dtypes.bfloat16)
        # keep = ~mask, transposed to [k, q], per key tile
        keepT = (~mask[b]).T
        keep_h = np.ascontiguousarray(keepT.reshape(KT, P, S)).astype(
            ml_dtypes.bfloat16
        )
        in_maps.append(
            {
                "xT": xT_h,
                "keep": keep_h,
                "wqk": wqk_h,
                "wv": wv_h,
                "bqk": bqk_h,
            }
        )

    nc = _get_nc()
    res = run_bass_kernel_spmd(nc, in_maps, core_ids=list(range(B)))
    raw = np.stack(
        [np.asarray(res.results[b]["out"]).astype(np.float32) for b in range(B)]
    )
    out = raw[:, :, :DOUT] / raw[:, :, DOUT:]  # normalize by the softmax denom
    out = out + bv[None, None, :]
    return out.astype(np.float32)
